# revision 2
# baseline (speedup 1.0000x reference)
"""GAT (3-layer, 4-head) + MLP head on 8 Trainium2 NeuronCores — v2.

Design: dst-shard nodes across 8 cores (6272-padded rows/core). Per layer a
bf16 table TAB[50176, 256] = [h(128) | as(4) | ad(4) | pad] lives in each
core's DRAM (local shard built by matmul, AllGather'd). Edges (with self
loops) are host-sorted per (core, 128-dst block), split into lo/hi src
halves for int16 dma_gather indices. Per block: two dma_gathers fetch the
full 512B src rows (queues 0/1), one dma_gather fetches the 256B [as|ad]
half of dst rows from the LOCAL shard (queue 2). p = exp(leaky_relu(as+ad))
per edge/head; V = [p*h | p]; one-hot M (from seg lane ids) aggregates V
per 128-dst block via matmul into PSUM; epilogue normalizes, applies bias,
and builds the next layer's table rows. MLP head runs on the core's shard.
"""
import numpy as np
import ml_dtypes

import concourse.bass as bass
import concourse.bacc as bacc
import concourse.mybir as mybir
import concourse.tile as tile
from concourse.masks import make_identity
from concourse.bass_utils import run_bass_kernel_spmd
from concourse import bass2jax


_run_cache = {}


def _make_runner(nc, n_cores):
    """Like bass2jax.run_bass_via_pjrt but with the jitted fn cached."""
    import jax
    import jax.numpy as jnp
    import concourse.mybir as _mybir
    bass2jax.install_neuronx_cc_hook()
    partition_name = (nc.partition_id_tensor.name
                      if nc.partition_id_tensor else None)
    in_names, out_names, out_avals, zero_shapes = [], [], [], []
    for alloc in nc.m.functions[0].allocations:
        if not isinstance(alloc, _mybir.MemoryLocationSet):
            continue
        name = alloc.memorylocations[0].name
        if alloc.kind == "ExternalInput":
            if name != partition_name:
                in_names.append(name)
        elif alloc.kind == "ExternalOutput":
            out_names.append(name)
            shape = tuple(alloc.tensor_shape)
            dtype = _mybir.dt.np(alloc.dtype)
            out_avals.append(jax.core.ShapedArray(shape, dtype))
            zero_shapes.append((shape, dtype))
    n_params = len(in_names)
    n_outs = len(out_avals)
    all_names = in_names + out_names + ([partition_name] if partition_name else [])
    donate = tuple(range(n_params, n_params + n_outs))

    def _body(*args):
        operands = list(args)
        if partition_name is not None:
            operands.append(bass2jax.partition_id_tensor())
        outs = bass2jax._bass_exec_p.bind(
            *operands, out_avals=tuple(out_avals), in_names=tuple(all_names),
            out_names=tuple(out_names), lowering_input_output_aliases=(),
            sim_require_finite=True, sim_require_nnan=True, nc=nc)
        return tuple(outs)

    devices = jax.devices()[:n_cores]
    mesh = bass2jax.Mesh(np.asarray(devices), ("core",))
    in_specs = (bass2jax.PartitionSpec("core"),) * (n_params + n_outs)
    out_specs = (bass2jax.PartitionSpec("core"),) * n_outs
    sharded = jax.jit(
        bass2jax.shard_map(_body, mesh=mesh, in_specs=in_specs,
                           out_specs=out_specs, check_rep=False),
        donate_argnums=donate, keep_unused=True)

    def run(in_maps):
        concat_in = [
            np.concatenate([np.asarray(in_maps[c][nm]) for c in range(n_cores)],
                           axis=0) for nm in in_names]
        concat_zeros = [np.zeros((n_cores * s0[0], *s0[1:]), dt)
                        for (s0, dt) in zero_shapes]
        out_arrs = sharded(*concat_in, *concat_zeros)
        return [
            {nm: np.asarray(out_arrs[i]).reshape(n_cores, *out_avals[i].shape)[c]
             for i, nm in enumerate(out_names)}
            for c in range(n_cores)]
    return run


def _run_spmd(nc, maps, _retries=2):
    key = id(nc)
    if key not in _run_cache:
        _run_cache[key] = _make_runner(nc, NCORES)
    for attempt in range(_retries + 1):
        try:
            return _run_cache[key](maps)
        except Exception:
            if attempt == _retries:
                raise
            import time as _t
            _t.sleep(5.0)

F32 = mybir.dt.float32
BF16 = mybir.dt.bfloat16
I16 = mybir.dt.int16
AF = mybir.ActivationFunctionType
BFNP = ml_dtypes.bfloat16

N, E, F = 50000, 1600000, 128
H, C, L = 4, 32, 3
NCORES = 8
NPC = N // NCORES            # 6250 real rows per core
NB = 49                      # 128-dst blocks per core
NPAD = NB * 128              # 6272 padded rows per core
NTOT = NCORES * NPAD         # 50176 padded table rows
SPLIT = 32768                # int16 index split
TW = 256                     # bf16 table row elems (512B)
NEG_SLOPE = 0.2

_cache = {}


def _cwrap(idx2d):
    """[B, n] -> [16, B*(n/16)]: value at (j%16, b*(n/16) + j//16) = idx[b, j]."""
    B, n = idx2d.shape
    assert n % 16 == 0
    w = idx2d.reshape(B, n // 16, 16).transpose(2, 0, 1)  # [16, B, n/16]
    return np.ascontiguousarray(w.reshape(16, B * (n // 16))).astype(np.int16)


def _host_prep(x, edge_index, W, att_src, att_dst, b_conv, W1, b1, W2, b2):
    src = np.concatenate([np.asarray(edge_index[0]), np.arange(N, dtype=np.int32)])
    dst = np.concatenate([np.asarray(edge_index[1]), np.arange(N, dtype=np.int32)])
    src = src.astype(np.int64)
    dst = dst.astype(np.int64)
    core = dst // NPC
    dloc = dst - core * NPC
    blk = dloc // 128
    seg = (dloc - blk * 128)
    gsrc = (src // NPC) * NPAD + (src % NPC)     # padded global src id
    hi = gsrc >= SPLIT

    key = ((core * NB + blk) * 2 + hi).astype(np.int32)
    order = np.argsort(key)
    gsrc, seg_s, key_s = gsrc[order], seg[order], key[order]
    counts = np.bincount(key_s, minlength=NCORES * NB * 2).reshape(NCORES, NB, 2)
    TLO = int(np.ceil(counts[:, :, 0].max() / 128))
    THI = int(np.ceil(counts[:, :, 1].max() / 128))
    TB = TLO + THI

    starts = np.concatenate([[0], np.cumsum(counts.reshape(-1))[:-1]])
    pos = np.arange(len(key_s)) - starts[key_s]
    grp = key_s >> 1
    is_lo = (key_s & 1) == 0
    is_hi = ~is_lo

    ilo = np.zeros(NCORES * NB * TLO * 128, np.int16)
    ihi = np.zeros(NCORES * NB * THI * 128, np.int16)
    iad = np.zeros(NCORES * NB * TB * 128, np.int16)
    segf = np.full(NCORES * NB * TB * 128, 200.0, np.float32)
    adv = (grp % NB) * 128 + seg_s     # local dst id = b*128 + seg
    ilo[grp[is_lo] * (TLO * 128) + pos[is_lo]] = gsrc[is_lo]
    ihi[grp[is_hi] * (THI * 128) + pos[is_hi]] = gsrc[is_hi] - SPLIT
    iad[grp * (TB * 128) + pos + np.where(is_lo, 0, TLO * 128)] = adv
    segf[grp * (TB * 128) + pos + np.where(is_lo, 0, TLO * 128)] = seg_s
    ilo = ilo.reshape(NCORES, NB, TLO * 128)
    ihi = ihi.reshape(NCORES, NB, THI * 128)
    iad = iad.reshape(NCORES, NB, TB * 128)
    segf = segf.reshape(NCORES, NB, TB * 128)

    # weights: Wc[l] = [W | W@Ss | W@Sd | 0] in bf16
    Wc = np.zeros((L, F, TW), np.float32)
    for l in range(L):
        Ss = np.zeros((F, H), np.float32)
        Sd = np.zeros((F, H), np.float32)
        for h in range(H):
            Ss[h * C:(h + 1) * C, h] = att_src[l, h]
            Sd[h * C:(h + 1) * C, h] = att_dst[l, h]
        Wc[l, :, :F] = W[l]
        Wc[l, :, F:F + H] = W[l] @ Ss
        Wc[l, :, F + H:F + 2 * H] = W[l] @ Sd
    biasm = np.broadcast_to(b_conv[:, None, :], (L, 128, F)).astype(np.float32)

    xT = np.zeros((NCORES, 128, NPAD), BFNP)
    xt_full = np.asarray(x).T.astype(BFNP)   # [128, N]
    wcb = Wc.astype(BFNP)
    bmb = biasm.astype(BFNP)
    w1af = np.ascontiguousarray(W1[:F]).astype(np.float32)
    w1bb = np.ascontiguousarray(W1[F:]).astype(BFNP)
    b1f = b1.reshape(-1, 1).astype(np.float32)
    w2f = np.ascontiguousarray(W2).astype(np.float32)
    b2f = np.asarray(b2, np.float32).reshape(1, 1)
    maps = []
    for k in range(NCORES):
        xT[k, :, :NPC] = xt_full[:, k * NPC:(k + 1) * NPC]
        parts = [np.ascontiguousarray(xT[k]),
                 _cwrap(ilo[k]), _cwrap(ihi[k]), _cwrap(iad[k]),
                 np.ascontiguousarray(
                     segf[k].reshape(NB, TB, 128).transpose(0, 2, 1)
                 ).astype(np.uint8),
                 wcb, bmb, w1af, w1bb, b1f, w2f, b2f]
        bufs = []
        for p in parts:
            b_ = p.reshape(-1).view(np.uint8)
            pad = (-len(b_)) % 64
            bufs.append(b_)
            if pad:
                bufs.append(np.zeros(pad, np.uint8))
        blob = np.concatenate(bufs)[None, :]
        maps.append({"blob": np.ascontiguousarray(blob)})
    return maps, TLO, THI


def _blob_offsets(TLO, THI):
    TB = TLO + THI
    shapes = [("x0T", 2, (128, NPAD)),
              ("ilo", 2, (16, NB * TLO * 8)),
              ("ihi", 2, (16, NB * THI * 8)),
              ("iad", 2, (16, NB * TB * 8)),
              ("segb", 1, (NB, 128, TB)),
              ("wc", 2, (L, 128, TW)),
              ("biasm", 2, (L, 128, F)),
              ("w1a", 4, (F, 32)),
              ("w1b", 2, (F, 32)),
              ("b1", 4, (32, 1)),
              ("w2", 4, (32, 1)),
              ("b2", 4, (1, 1))]
    offs = {}
    o = 0
    for name, sz, sh in shapes:
        nb = sz * int(np.prod(sh))
        offs[name] = (o, sh)
        o += nb + ((-nb) % 64)
    return offs, o


def _build(TLO, THI):
    TB = TLO + THI
    nc = bacc.Bacc("TRN2", target_bir_lowering=False, debug=False,
                   num_devices=NCORES, num_swdge_queues=4)
    offs, BYTES = _blob_offsets(TLO, THI)
    d_blob = nc.dram_tensor("blob", [1, BYTES], mybir.dt.uint8,
                            kind="ExternalInput")
    _dt_for = {"x0T": BF16, "ilo": I16, "ihi": I16, "iad": I16,
               "segb": mybir.dt.uint8, "wc": BF16, "biasm": BF16,
               "w1a": F32, "w1b": BF16, "b1": F32, "w2": F32, "b2": F32}
    _sz = {BF16: 2, I16: 2, F32: 4, mybir.dt.uint8: 1}

    def bview(name):
        o, sh = offs[name]
        mdt = _dt_for[name]
        nb = _sz[mdt] * int(np.prod(sh))
        ap = d_blob[0:1, o:o + nb].bitcast(mdt)   # [1, n_elems]
        if len(sh) == 2:
            return ap.rearrange("a (p c) -> p (a c)", p=sh[0])
        return ap.rearrange("a (b p c) -> b (a p) c", b=sh[0], p=sh[1])

    d_x0T, d_ilo, d_ihi, d_iad = (bview("x0T"), bview("ilo"), bview("ihi"),
                                  bview("iad"))
    d_seg, d_wc, d_bm = bview("segb"), bview("wc"), bview("biasm")
    d_w1a, d_w1b, d_b1 = bview("w1a"), bview("w1b"), bview("b1")
    d_w2, d_b2 = bview("w2"), bview("b2")
    r_ilo = nc.dram_tensor("r_ilo", [128, NB * TLO * 8], I16, kind="Internal")
    r_ihi = nc.dram_tensor("r_ihi", [128, NB * THI * 8], I16, kind="Internal")
    r_iad = nc.dram_tensor("r_iad", [128, NB * TB * 8], I16, kind="Internal")
    d_out = nc.dram_tensor("out", [NPC, 1], F32, kind="ExternalOutput")

    tabl = [nc.dram_tensor(f"tabl{i}", [NPAD, TW], BF16, kind="Internal")
            for i in range(L)]
    tabg = [nc.dram_tensor(f"tabg{i}", [NTOT, TW], BF16, kind="Internal",
                           addr_space="Shared") for i in range(L)]
    d_x3 = nc.dram_tensor("x3T", [128, NPAD], F32, kind="Internal")

    with tile.TileContext(nc) as tc:
        with tc.tile_pool(name="const", bufs=1) as cp, \
             tc.tile_pool(name="sb", bufs=2) as sb, \
             tc.tile_pool(name="ps", bufs=2, space="PSUM") as ps, \
             tc.tile_pool(name="ps1", bufs=2, space="PSUM") as ps1:
            wc_s = cp.tile([128, L * TW], BF16, tag="wc")
            nc.sync.dma_start(wc_s[:].rearrange("p (l c) -> p l c", l=L),
                              d_wc.rearrange("l p c -> p l c"))
            bm_s = cp.tile([128, L * F], BF16, tag="bm")
            nc.sync.dma_start(bm_s[:].rearrange("p (l c) -> p l c", l=L),
                              d_bm.rearrange("l p c -> p l c"))
            iota_s = cp.tile([128, 128], BF16, tag="iota")
            nc.gpsimd.iota(iota_s[:], pattern=[[1, 128]], base=0,
                           channel_multiplier=0,
                           allow_small_or_imprecise_dtypes=True)
            id_s = cp.tile([128, 128], F32, tag="id")
            make_identity(nc, id_s[:])
            w1a_s = cp.tile([F, 32], F32, tag="w1a")
            nc.sync.dma_start(w1a_s[:], d_w1a)
            w1b_s = cp.tile([F, 32], BF16, tag="w1b")
            nc.sync.dma_start(w1b_s[:], d_w1b)
            b1_s = cp.tile([32, 1], F32, tag="b1")
            nc.sync.dma_start(b1_s[:], d_b1)
            w2_s = cp.tile([32, 1], F32, tag="w2")
            nc.sync.dma_start(w2_s[:], d_w2)
            b2_s = cp.tile([1, 1], F32, tag="b2")
            nc.sync.dma_start(b2_s[:], d_b2)
            x0_s = cp.tile([128, NPAD], BF16, tag="x0")
            nc.sync.dma_start(x0_s[:], d_x0T)

            # ---- phase 0: replicate compact idx to all 8 partition groups ----
            for d_c, d_r, w in ((d_ilo, r_ilo, NB * TLO * 8),
                                (d_ihi, r_ihi, NB * THI * 8),
                                (d_iad, r_iad, NB * TB * 8)):
                st = cp.tile([16, w], I16, tag="idxstage")
                nc.sync.dma_start(st[:], d_c)
                for gq in range(8):
                    nc.sync.dma_start(d_r[16 * gq:16 * gq + 16, :], st[:])

            # ---- phase 1: local table shard for layer 0 ----
            for i in range(NB):
                pt = ps1.tile([128, TW], F32, tag="tabp")
                nc.tensor.matmul(pt[:], x0_s[:, i * 128:(i + 1) * 128],
                                 wc_s[:, 0:TW], start=True, stop=True)
                ts_ = sb.tile([128, TW], BF16, tag="tabs")
                nc.vector.tensor_copy(ts_[:], pt[:])
                nc.sync.dma_start(tabl[0][i * 128:(i + 1) * 128, :], ts_[:])
            tc.strict_bb_all_engine_barrier()
            nc.gpsimd.collective_compute(
                "AllGather", mybir.AluOpType.bypass,
                ins=[tabl[0][:]], outs=[tabg[0][:]],
                replica_groups=[list(range(NCORES))])
            tc.strict_bb_all_engine_barrier()

            # ---- phase 2: layers ----
            for l in range(L):
                with tc.For_i(0, NB, 1) as i:
                    ilo_s = sb.tile([128, TLO * 8], I16, tag="ilo")
                    nc.sync.dma_start(
                        ilo_s[:], r_ilo[:, bass.ds(i * (TLO * 8), TLO * 8)])
                    ihi_s = sb.tile([128, THI * 8], I16, tag="ihi")
                    nc.sync.dma_start(
                        ihi_s[:], r_ihi[:, bass.ds(i * (THI * 8), THI * 8)])
                    iad_s = sb.tile([128, TB * 8], I16, tag="iad")
                    nc.sync.dma_start(
                        iad_s[:], r_iad[:, bass.ds(i * (TB * 8), TB * 8)])
                    seg8 = sb.tile([128, TB], mybir.dt.uint8, tag="seg8")
                    nc.sync.dma_start(seg8[:], d_seg[i])
                    seg_s = sb.tile([128, TB], BF16, tag="seg")
                    nc.vector.tensor_copy(seg_s[:], seg8[:])

                    g = sb.tile([128, TB * TW], BF16, tag="g")
                    nc.gpsimd.dma_gather(
                        out_ap=g[:, :TLO * TW].rearrange("p (t e) -> p t e", t=TLO),
                        in_ap=tabg[l][0:SPLIT, :],
                        idxs_ap=ilo_s[:],
                        num_idxs=TLO * 128, num_idxs_reg=TLO * 128,
                        elem_size=TW, single_packet=False, queue_num=0)
                    nc.gpsimd.dma_gather(
                        out_ap=g[:, TLO * TW:].rearrange("p (t e) -> p t e", t=THI),
                        in_ap=tabg[l][SPLIT:NTOT, :],
                        idxs_ap=ihi_s[:],
                        num_idxs=THI * 128, num_idxs_reg=THI * 128,
                        elem_size=TW, single_packet=False, queue_num=1)
                    ga = sb.tile([128, TB * 128], BF16, tag="ga")
                    nc.gpsimd.dma_gather(
                        out_ap=ga[:].rearrange("p (t e) -> p t e", t=TB),
                        in_ap=tabl[l][:, 128:256],
                        idxs_ap=iad_s[:],
                        num_idxs=TB * 128, num_idxs_reg=TB * 128,
                        elem_size=128, elem_step=TW,
                        single_packet=False, queue_num=2)

                    gv = g[:].rearrange("p (t e) -> p t e", t=TB)
                    gav = ga[:].rearrange("p (t e) -> p t e", t=TB)
                    z = sb.tile([128, TB * H], BF16, tag="z")
                    nc.vector.tensor_add(
                        z[:].rearrange("p (t h) -> p t h", t=TB),
                        gv[:, :, F:F + H], gav[:, :, H:2 * H])
                    zl = sb.tile([128, TB * H], BF16, tag="zl")
                    nc.scalar.activation(zl[:], z[:], AF.Lrelu, alpha=NEG_SLOPE)
                    p = sb.tile([128, TB * H], BF16, tag="p")
                    nc.scalar.activation(p[:], zl[:], AF.Exp)

                    V = sb.tile([128, TB * (F + H)], BF16, tag="V")
                    Vv = V[:].rearrange("p (t e) -> p t e", t=TB)
                    nc.vector.tensor_mul(
                        Vv[:, :, 0:F].rearrange("p t (h c) -> p t h c", h=H),
                        gv[:, :, 0:F].rearrange("p t (h c) -> p t h c", h=H),
                        p[:].rearrange("p (t h) -> p t h", t=TB).unsqueeze(3)
                           .to_broadcast([128, TB, H, C]))
                    nc.vector.tensor_copy(
                        Vv[:, :, F:F + H],
                        p[:].rearrange("p (t h) -> p t h", t=TB))
                    M = sb.tile([128, TB * 128], BF16, tag="M")
                    nc.vector.tensor_tensor(
                        M[:].rearrange("p (t e) -> p t e", t=TB),
                        seg_s[:].unsqueeze(2).to_broadcast([128, TB, 128]),
                        iota_s[:].unsqueeze(1).to_broadcast([128, TB, 128]),
                        op=mybir.AluOpType.is_equal)

                    pb = ps.tile([128, F + H], F32, tag="acc")
                    for t in range(TB):
                        nc.tensor.matmul(
                            pb[:], M[:, t * 128:(t + 1) * 128],
                            V[:, t * (F + H):(t + 1) * (F + H)],
                            start=(t == 0), stop=(t == TB - 1))

                    rec = sb.tile([128, H], F32, tag="rec")
                    nc.vector.reciprocal(rec[:], pb[:, F:F + H])
                    xb = sb.tile([128, F], F32, tag="xb")
                    nc.vector.tensor_mul(
                        xb[:].rearrange("p (h c) -> p h c", h=H),
                        pb[:, 0:F].rearrange("p (h c) -> p h c", h=H),
                        rec[:].unsqueeze(2).to_broadcast([128, H, C]))
                    xb2 = sb.tile([128, F], F32, tag="xb2")
                    nc.vector.tensor_add(xb2[:], xb[:],
                                         bm_s[:, l * F:(l + 1) * F])
                    xtp = ps1.tile([128, 128], F32, tag="xtp")
                    nc.tensor.transpose(xtp[:], xb2[:], id_s[:])
                    if l < L - 1:
                        xbT = sb.tile([128, 128], BF16, tag="xbT")
                        nc.vector.tensor_copy(xbT[:], xtp[:])
                        tbp = ps1.tile([128, TW], F32, tag="tabp2")
                        nc.tensor.matmul(tbp[:], xbT[:],
                                         wc_s[:, (l + 1) * TW:(l + 2) * TW],
                                         start=True, stop=True)
                        tbs = sb.tile([128, TW], BF16, tag="tabs2")
                        nc.vector.tensor_copy(tbs[:], tbp[:])
                        nc.sync.dma_start(
                            tabl[l + 1][bass.ds(i * 128, 128), :], tbs[:])
                    else:
                        x3s = sb.tile([128, 128], F32, tag="x3s")
                        nc.vector.tensor_copy(x3s[:], xtp[:])
                        nc.sync.dma_start(d_x3[:, bass.ds(i * 128, 128)], x3s[:])
                tc.strict_bb_all_engine_barrier()
                if l < L - 1:
                    nc.gpsimd.collective_compute(
                        "AllGather", mybir.AluOpType.bypass,
                        ins=[tabl[l + 1][:]], outs=[tabg[l + 1][:]],
                        replica_groups=[list(range(NCORES))])
                    tc.strict_bb_all_engine_barrier()

            # ---- phase 3: MLP ----
            x3_s = cp.tile([128, NPAD], F32, tag="x3")
            nc.sync.dma_start(x3_s[:], d_x3[:])
            orow = cp.tile([1, NPAD], F32, tag="orow")
            nchunk = (NPAD + 511) // 512
            for i in range(nchunk):
                base = i * 512
                cnt = min(512, NPAD - base)
                hp = ps1.tile([32, 512], F32, tag="tabp")
                nc.tensor.matmul(hp[:, :cnt], w1a_s[:],
                                 x3_s[:, base:base + cnt], start=True, stop=False)
                nc.tensor.matmul(hp[:, :cnt], w1b_s[:],
                                 x0_s[:, base:base + cnt], start=False, stop=True)
                h1 = sb.tile([32, 512], F32, tag="h1")
                nc.scalar.activation(h1[:, :cnt], hp[:, :cnt], AF.Relu,
                                     bias=b1_s[:])
                op_ = ps1.tile([1, 512], F32, tag="xtp")
                nc.tensor.matmul(op_[:, :cnt], w2_s[:], h1[:32, :cnt],
                                 start=True, stop=True)
                nc.scalar.activation(orow[0:1, base:base + cnt], op_[:, :cnt],
                                     AF.Sigmoid, bias=b2_s[:])
            nc.sync.dma_start(d_out[:].rearrange("n one -> one n"),
                              orow[0:1, :NPC])
    nc.compile()
    return nc


def kernel(**inputs):
    maps, TLO, THI = _host_prep(**inputs)
    key = (TLO, THI)
    if key not in _cache:
        _cache[key] = _build(TLO, THI)
    nc = _cache[key]
    results = _run_spmd(nc, maps)
    out = np.concatenate([results[k]["out"] for k in range(NCORES)], axis=0)
    return out


def run_traced(**inputs):
    """Returns (out, best_ns) — min wall-clock over repeated steady-state runs."""
    import time
    maps, TLO, THI = _host_prep(**inputs)
    key = (TLO, THI)
    if key not in _cache:
        _cache[key] = _build(TLO, THI)
    nc = _cache[key]
    results = _run_spmd(nc, maps)  # warm (compile)
    best = None
    for _ in range(3):
        t0 = time.perf_counter()
        results = _run_spmd(nc, maps)
        dt = time.perf_counter() - t0
        best = dt if best is None else min(best, dt)
    out = np.concatenate([results[k]["out"] for k in range(NCORES)], axis=0)
    return out, int(best * 1e9)


# revision 4
# speedup vs baseline: 1.5944x; 1.5944x over previous
"""GAT (3-layer, 4-head) + MLP head on 8 Trainium2 NeuronCores — v2.

Design: dst-shard nodes across 8 cores (6272-padded rows/core). Per layer a
bf16 table TAB[50176, 256] = [h(128) | as(4) | ad(4) | pad] lives in each
core's DRAM (local shard built by matmul, AllGather'd). Edges (with self
loops) are host-sorted per (core, 128-dst block), split into lo/hi src
halves for int16 dma_gather indices. Per block: two dma_gathers fetch the
full 512B src rows (queues 0/1), one dma_gather fetches the 256B [as|ad]
half of dst rows from the LOCAL shard (queue 2). p = exp(leaky_relu(as+ad))
per edge/head; V = [p*h | p]; one-hot M (from seg lane ids) aggregates V
per 128-dst block via matmul into PSUM; epilogue normalizes, applies bias,
and builds the next layer's table rows. MLP head runs on the core's shard.
"""
import numpy as np
import ml_dtypes

import concourse.bass as bass
import concourse.bacc as bacc
import concourse.mybir as mybir
import concourse.tile as tile
from concourse.masks import make_identity
from concourse.bass_utils import run_bass_kernel_spmd
from concourse import bass2jax


_run_cache = {}


def _make_runner(nc, n_cores):
    """Like bass2jax.run_bass_via_pjrt but with the jitted fn cached."""
    import jax
    import jax.numpy as jnp
    import concourse.mybir as _mybir
    bass2jax.install_neuronx_cc_hook()
    partition_name = (nc.partition_id_tensor.name
                      if nc.partition_id_tensor else None)
    in_names, out_names, out_avals, zero_shapes = [], [], [], []
    for alloc in nc.m.functions[0].allocations:
        if not isinstance(alloc, _mybir.MemoryLocationSet):
            continue
        name = alloc.memorylocations[0].name
        if alloc.kind == "ExternalInput":
            if name != partition_name:
                in_names.append(name)
        elif alloc.kind == "ExternalOutput":
            out_names.append(name)
            shape = tuple(alloc.tensor_shape)
            dtype = _mybir.dt.np(alloc.dtype)
            out_avals.append(jax.core.ShapedArray(shape, dtype))
            zero_shapes.append((shape, dtype))
    n_params = len(in_names)
    n_outs = len(out_avals)
    all_names = in_names + out_names + ([partition_name] if partition_name else [])
    donate = tuple(range(n_params, n_params + n_outs))

    def _body(*args):
        operands = list(args)
        if partition_name is not None:
            operands.append(bass2jax.partition_id_tensor())
        outs = bass2jax._bass_exec_p.bind(
            *operands, out_avals=tuple(out_avals), in_names=tuple(all_names),
            out_names=tuple(out_names), lowering_input_output_aliases=(),
            sim_require_finite=True, sim_require_nnan=True, nc=nc)
        return tuple(outs)

    devices = jax.devices()[:n_cores]
    mesh = bass2jax.Mesh(np.asarray(devices), ("core",))
    in_specs = (bass2jax.PartitionSpec("core"),) * (n_params + n_outs)
    out_specs = (bass2jax.PartitionSpec("core"),) * n_outs
    sharded = jax.jit(
        bass2jax.shard_map(_body, mesh=mesh, in_specs=in_specs,
                           out_specs=out_specs, check_rep=False),
        donate_argnums=donate, keep_unused=True)

    def run(in_maps):
        concat_in = [
            np.concatenate([np.asarray(in_maps[c][nm]) for c in range(n_cores)],
                           axis=0) for nm in in_names]
        concat_zeros = [np.zeros((n_cores * s0[0], *s0[1:]), dt)
                        for (s0, dt) in zero_shapes]
        out_arrs = sharded(*concat_in, *concat_zeros)
        return [
            {nm: np.asarray(out_arrs[i]).reshape(n_cores, *out_avals[i].shape)[c]
             for i, nm in enumerate(out_names)}
            for c in range(n_cores)]
    return run


def _run_spmd(nc, maps, _retries=2):
    key = id(nc)
    if key not in _run_cache:
        _run_cache[key] = _make_runner(nc, NCORES)
    for attempt in range(_retries + 1):
        try:
            return _run_cache[key](maps)
        except Exception:
            if attempt == _retries:
                raise
            import time as _t
            _t.sleep(5.0)

F32 = mybir.dt.float32
BF16 = mybir.dt.bfloat16
I16 = mybir.dt.int16
AF = mybir.ActivationFunctionType
BFNP = ml_dtypes.bfloat16
FP8NP = ml_dtypes.float8_e4m3fn
FP8 = mybir.dt.float8e4

N, E, F = 50000, 1600000, 128
H, C, L = 4, 32, 3
NCORES = 8
NPC = N // NCORES            # 6250 real rows per core
NB = 49                      # 128-dst blocks per core
NPAD = NB * 128              # 6272 padded rows per core
NTOT = NCORES * NPAD         # 50176 padded table rows
SPLIT = 32768                # int16 index split
TW = 256                     # bf16 table row elems (512B)
NEG_SLOPE = 0.2

_cache = {}


def _cwrap(idx2d):
    """[B, n] -> [16, B*(n/16)]: value at (j%16, b*(n/16) + j//16) = idx[b, j]."""
    B, n = idx2d.shape
    assert n % 16 == 0
    w = idx2d.reshape(B, n // 16, 16).transpose(2, 0, 1)  # [16, B, n/16]
    return np.ascontiguousarray(w.reshape(16, B * (n // 16))).astype(np.int16)


def _host_prep(x, edge_index, W, att_src, att_dst, b_conv, W1, b1, W2, b2):
    src = np.concatenate([np.asarray(edge_index[0]), np.arange(N, dtype=np.int32)])
    dst = np.concatenate([np.asarray(edge_index[1]), np.arange(N, dtype=np.int32)])
    src = src.astype(np.int64)
    dst = dst.astype(np.int64)
    core = dst // NPC
    dloc = dst - core * NPC
    blk = dloc // 128
    seg = (dloc - blk * 128)
    gsrc = (src // NPC) * NPAD + (src % NPC)     # padded global src id
    hi = gsrc >= SPLIT

    key = ((core * NB + blk) * 2 + hi).astype(np.int32)
    order = np.argsort(key)
    gsrc, seg_s, key_s = gsrc[order], seg[order], key[order]
    counts = np.bincount(key_s, minlength=NCORES * NB * 2).reshape(NCORES, NB, 2)
    TLO = int(np.ceil(counts[:, :, 0].max() / 128))
    THI = int(np.ceil(counts[:, :, 1].max() / 128))
    TB = TLO + THI

    starts = np.concatenate([[0], np.cumsum(counts.reshape(-1))[:-1]])
    pos = np.arange(len(key_s)) - starts[key_s]
    grp = key_s >> 1
    is_lo = (key_s & 1) == 0
    is_hi = ~is_lo

    ilo = np.zeros(NCORES * NB * TLO * 128, np.int16)
    ihi = np.zeros(NCORES * NB * THI * 128, np.int16)
    segf = np.full(NCORES * NB * TB * 128, 200.0, np.float32)
    ilo[grp[is_lo] * (TLO * 128) + pos[is_lo]] = gsrc[is_lo]
    ihi[grp[is_hi] * (THI * 128) + pos[is_hi]] = gsrc[is_hi] - SPLIT
    segf[grp * (TB * 128) + pos + np.where(is_lo, 0, TLO * 128)] = seg_s
    ilo = ilo.reshape(NCORES, NB, TLO * 128)
    ihi = ihi.reshape(NCORES, NB, THI * 128)
    segf = segf.reshape(NCORES, NB, TB * 128)

    # weights: Wc[l] = [W | W@Ss | W@Sd | 0] in bf16
    Wc = np.zeros((L, F, TW), np.float32)
    for l in range(L):
        Ss = np.zeros((F, H), np.float32)
        Sd = np.zeros((F, H), np.float32)
        for h in range(H):
            Ss[h * C:(h + 1) * C, h] = att_src[l, h]
            Sd[h * C:(h + 1) * C, h] = att_dst[l, h]
        Wc[l, :, :F] = W[l]
        Wc[l, :, F:F + H] = W[l] @ Ss
        Wc[l, :, F + H:F + 2 * H] = W[l] @ Sd
    biasm = np.broadcast_to(b_conv[:, None, :], (L, 128, F)).astype(np.float32)

    xT = np.zeros((NCORES, 128, NPAD), FP8NP)
    xt_full = np.asarray(x).T.astype(FP8NP)   # [128, N]
    wcb = Wc.astype(BFNP)
    bmb = biasm.astype(BFNP)
    w1af = np.ascontiguousarray(W1[:F]).astype(np.float32)
    w1bb = np.ascontiguousarray(W1[F:]).astype(BFNP)
    b1f = b1.reshape(-1, 1).astype(np.float32)
    w2f = np.ascontiguousarray(W2).astype(np.float32)
    b2f = np.asarray(b2, np.float32).reshape(1, 1)
    maps = []
    for k in range(NCORES):
        xT[k, :, :NPC] = xt_full[:, k * NPC:(k + 1) * NPC]
        parts = [np.ascontiguousarray(xT[k]),
                 _cwrap(ilo[k]), _cwrap(ihi[k]),
                 np.ascontiguousarray(
                     segf[k].reshape(NB, TB, 128).transpose(0, 2, 1)
                 ).astype(np.uint8),
                 wcb, bmb, w1af, w1bb, b1f, w2f, b2f]
        bufs = []
        for p in parts:
            b_ = p.reshape(-1).view(np.uint8)
            pad = (-len(b_)) % 64
            bufs.append(b_)
            if pad:
                bufs.append(np.zeros(pad, np.uint8))
        blob = np.concatenate(bufs)[None, :]
        maps.append({"blob": np.ascontiguousarray(blob)})
    return maps, TLO, THI


def _blob_offsets(TLO, THI):
    TB = TLO + THI
    shapes = [("x0T", 1, (128, NPAD)),
              ("ilo", 2, (16, NB * TLO * 8)),
              ("ihi", 2, (16, NB * THI * 8)),
              ("segb", 1, (NB, 128, TB)),
              ("wc", 2, (L, 128, TW)),
              ("biasm", 2, (L, 128, F)),
              ("w1a", 4, (F, 32)),
              ("w1b", 2, (F, 32)),
              ("b1", 4, (32, 1)),
              ("w2", 4, (32, 1)),
              ("b2", 4, (1, 1))]
    offs = {}
    o = 0
    for name, sz, sh in shapes:
        nb = sz * int(np.prod(sh))
        offs[name] = (o, sh)
        o += nb + ((-nb) % 64)
    return offs, o


def _build(TLO, THI):
    TB = TLO + THI
    nc = bacc.Bacc("TRN2", target_bir_lowering=False, debug=False,
                   num_devices=NCORES, num_swdge_queues=4)
    offs, BYTES = _blob_offsets(TLO, THI)
    d_blob = nc.dram_tensor("blob", [1, BYTES], mybir.dt.uint8,
                            kind="ExternalInput")
    _dt_for = {"x0T": FP8, "ilo": I16, "ihi": I16,
               "segb": mybir.dt.uint8, "wc": BF16, "biasm": BF16,
               "w1a": F32, "w1b": BF16, "b1": F32, "w2": F32, "b2": F32}
    _sz = {BF16: 2, I16: 2, F32: 4, mybir.dt.uint8: 1, FP8: 1}

    def bview(name):
        o, sh = offs[name]
        mdt = _dt_for[name]
        nb = _sz[mdt] * int(np.prod(sh))
        ap = d_blob[0:1, o:o + nb].bitcast(mdt)   # [1, n_elems]
        if len(sh) == 2:
            return ap.rearrange("a (p c) -> p (a c)", p=sh[0])
        return ap.rearrange("a (b p c) -> b (a p) c", b=sh[0], p=sh[1])

    d_x0T, d_ilo, d_ihi = bview("x0T"), bview("ilo"), bview("ihi")
    d_seg, d_wc, d_bm = bview("segb"), bview("wc"), bview("biasm")
    d_w1a, d_w1b, d_b1 = bview("w1a"), bview("w1b"), bview("b1")
    d_w2, d_b2 = bview("w2"), bview("b2")
    r_ilo = nc.dram_tensor("r_ilo", [128, NB * TLO * 8], I16, kind="Internal")
    r_ihi = nc.dram_tensor("r_ihi", [128, NB * THI * 8], I16, kind="Internal")
    d_out = nc.dram_tensor("out", [NPC, 1], F32, kind="ExternalOutput")

    tabl = [nc.dram_tensor(f"tabl{i}", [NPAD, TW], BF16, kind="Internal")
            for i in range(L)]
    tabg = [nc.dram_tensor(f"tabg{i}", [NTOT, TW], BF16, kind="Internal",
                           addr_space="Shared") for i in range(L)]
    d_x3 = nc.dram_tensor("x3T", [128, NPAD], F32, kind="Internal")

    with tile.TileContext(nc) as tc:
        with tc.tile_pool(name="const", bufs=1) as cp, \
             tc.tile_pool(name="sb", bufs=2) as sb, \
             tc.tile_pool(name="ps", bufs=2, space="PSUM") as ps, \
             tc.tile_pool(name="ps1", bufs=2, space="PSUM") as ps1, \
             tc.tile_pool(name="ps2", bufs=1, space="PSUM") as ps2:
            wc_s = cp.tile([128, L * TW], BF16, tag="wc")
            nc.sync.dma_start(wc_s[:].rearrange("p (l c) -> p l c", l=L),
                              d_wc.rearrange("l p c -> p l c"))
            bm_s = cp.tile([128, L * F], BF16, tag="bm")
            nc.sync.dma_start(bm_s[:].rearrange("p (l c) -> p l c", l=L),
                              d_bm.rearrange("l p c -> p l c"))
            iota_s = cp.tile([128, 128], BF16, tag="iota")
            nc.gpsimd.iota(iota_s[:], pattern=[[1, 128]], base=0,
                           channel_multiplier=0,
                           allow_small_or_imprecise_dtypes=True)
            id_s = cp.tile([128, 128], F32, tag="id")
            make_identity(nc, id_s[:])
            idb_s = cp.tile([128, 128], BF16, tag="idb")
            nc.vector.tensor_copy(idb_s[:], id_s[:])
            w1a_s = cp.tile([F, 32], F32, tag="w1a")
            nc.sync.dma_start(w1a_s[:], d_w1a)
            w1b_s = cp.tile([F, 32], BF16, tag="w1b")
            nc.sync.dma_start(w1b_s[:], d_w1b)
            b1_s = cp.tile([32, 1], F32, tag="b1")
            nc.sync.dma_start(b1_s[:], d_b1)
            w2_s = cp.tile([32, 1], F32, tag="w2")
            nc.sync.dma_start(w2_s[:], d_w2)
            b2_s = cp.tile([1, 1], F32, tag="b2")
            nc.sync.dma_start(b2_s[:], d_b2)
            x0_s = cp.tile([128, NPAD], BF16, tag="x0")
            nc.gpsimd.dma_start(x0_s[:], d_x0T)

            # ---- phase 0: replicate compact idx to all 8 partition groups ----
            for d_c, d_r, w in ((d_ilo, r_ilo, NB * TLO * 8),
                                (d_ihi, r_ihi, NB * THI * 8)):
                st = cp.tile([16, w], I16, tag="idxstage")
                nc.sync.dma_start(st[:], d_c)
                for gq in range(8):
                    nc.sync.dma_start(d_r[16 * gq:16 * gq + 16, :], st[:])

            # ---- phase 1: local table shard for layer 0 ----
            for i in range(NB):
                pt = ps1.tile([128, TW], F32, tag="xtp")
                nc.tensor.matmul(pt[:], x0_s[:, i * 128:(i + 1) * 128],
                                 wc_s[:, 0:TW], start=True, stop=True)
                ts_ = sb.tile([128, TW], BF16, tag="tabs")
                nc.vector.tensor_copy(ts_[:], pt[:])
                nc.sync.dma_start(tabl[0][i * 128:(i + 1) * 128, :], ts_[:])
            tc.strict_bb_all_engine_barrier()
            nc.gpsimd.collective_compute(
                "AllGather", mybir.AluOpType.bypass,
                ins=[tabl[0][:]], outs=[tabg[0][:]],
                replica_groups=[list(range(NCORES))])
            tc.strict_bb_all_engine_barrier()

            # ---- phase 2: layers ----
            for l in range(L):
                with tc.For_i(0, NB, 1) as i:
                    ilo_s = sb.tile([128, TLO * 8], I16, tag="ilo")
                    nc.sync.dma_start(
                        ilo_s[:], r_ilo[:, bass.ds(i * (TLO * 8), TLO * 8)])
                    ihi_s = sb.tile([128, THI * 8], I16, tag="ihi")
                    nc.sync.dma_start(
                        ihi_s[:], r_ihi[:, bass.ds(i * (THI * 8), THI * 8)])
                    seg8 = sb.tile([128, TB], mybir.dt.uint8, tag="seg8")
                    nc.sync.dma_start(seg8[:], d_seg[i])
                    seg_s = sb.tile([128, TB], BF16, tag="seg")
                    nc.vector.tensor_copy(seg_s[:], seg8[:])

                    g = sb.tile([128, TB * TW], BF16, tag="g")
                    nc.gpsimd.dma_gather(
                        out_ap=g[:, :TLO * TW].rearrange("p (t e) -> p t e", t=TLO),
                        in_ap=tabg[l][0:SPLIT, :],
                        idxs_ap=ilo_s[:],
                        num_idxs=TLO * 128, num_idxs_reg=TLO * 128,
                        elem_size=TW, single_packet=False, queue_num=0)
                    nc.gpsimd.dma_gather(
                        out_ap=g[:, TLO * TW:].rearrange("p (t e) -> p t e", t=THI),
                        in_ap=tabg[l][SPLIT:NTOT, :],
                        idxs_ap=ihi_s[:],
                        num_idxs=THI * 128, num_idxs_reg=THI * 128,
                        elem_size=TW, single_packet=False, queue_num=1)
                    adc = sb.tile([128, H], BF16, tag="adc")
                    nc.sync.dma_start(
                        adc[:], tabl[l][bass.ds(i * 128, 128), F + H:F + 2 * H])

                    gv = g[:].rearrange("p (t e) -> p t e", t=TB)
                    M = sb.tile([128, TB * 128], BF16, tag="M")
                    nc.vector.tensor_tensor(
                        M[:].rearrange("p (t e) -> p t e", t=TB),
                        seg_s[:].unsqueeze(2).to_broadcast([128, TB, 128]),
                        iota_s[:].unsqueeze(1).to_broadcast([128, TB, 128]),
                        op=mybir.AluOpType.is_equal)
                    pad = ps2.tile([128, TB * H], F32, tag="pad")
                    mts = sb.tile([128, TB * 128], BF16, tag="mts")
                    for t in range(TB):
                        mtp = ps2.tile([128, 128], BF16, tag="mtp")
                        nc.tensor.transpose(mtp[:], M[:, t * 128:(t + 1) * 128],
                                            idb_s[:])
                        nc.vector.tensor_copy(
                            mts[:, t * 128:(t + 1) * 128], mtp[:])
                        nc.tensor.matmul(
                            pad[:, t * H:(t + 1) * H],
                            mts[:, t * 128:(t + 1) * 128], adc[:],
                            start=True, stop=True)
                    z = sb.tile([128, TB * H], BF16, tag="z")
                    nc.vector.tensor_add(
                        z[:].rearrange("p (t h) -> p t h", t=TB),
                        gv[:, :, F:F + H],
                        pad[:].rearrange("p (t h) -> p t h", t=TB))
                    zl = sb.tile([128, TB * H], BF16, tag="zl")
                    nc.scalar.activation(zl[:], z[:], AF.Lrelu, alpha=NEG_SLOPE)
                    p = sb.tile([128, TB * H], BF16, tag="p")
                    nc.scalar.activation(p[:], zl[:], AF.Exp)

                    V = sb.tile([128, TB * (F + H)], BF16, tag="V")
                    Vv = V[:].rearrange("p (t e) -> p t e", t=TB)
                    nc.vector.tensor_mul(
                        Vv[:, :, 0:F].rearrange("p t (h c) -> p t h c", h=H),
                        gv[:, :, 0:F].rearrange("p t (h c) -> p t h c", h=H),
                        p[:].rearrange("p (t h) -> p t h", t=TB).unsqueeze(3)
                           .to_broadcast([128, TB, H, C]))
                    nc.vector.tensor_copy(
                        Vv[:, :, F:F + H],
                        p[:].rearrange("p (t h) -> p t h", t=TB))
                    pb = ps.tile([128, F + H], F32, tag="acc")
                    for t in range(TB):
                        nc.tensor.matmul(
                            pb[:], M[:, t * 128:(t + 1) * 128],
                            V[:, t * (F + H):(t + 1) * (F + H)],
                            start=(t == 0), stop=(t == TB - 1))

                    pse = sb.tile([128, H], F32, tag="pse")
                    nc.vector.tensor_scalar_add(pse[:], pb[:, F:F + H], 1e-20)
                    rec = sb.tile([128, H], F32, tag="rec")
                    nc.vector.reciprocal(rec[:], pse[:])
                    xb = sb.tile([128, F], F32, tag="xb")
                    nc.vector.tensor_mul(
                        xb[:].rearrange("p (h c) -> p h c", h=H),
                        pb[:, 0:F].rearrange("p (h c) -> p h c", h=H),
                        rec[:].unsqueeze(2).to_broadcast([128, H, C]))
                    xb2 = sb.tile([128, F], F32, tag="xb2")
                    nc.vector.tensor_add(xb2[:], xb[:],
                                         bm_s[:, l * F:(l + 1) * F])
                    xtp = ps1.tile([128, 128], F32, tag="xtp")
                    nc.tensor.transpose(xtp[:], xb2[:], id_s[:])
                    if l < L - 1:
                        xbT = sb.tile([128, 128], BF16, tag="xbT")
                        nc.vector.tensor_copy(xbT[:], xtp[:])
                        tbp = ps1.tile([128, TW], F32, tag="tabp2")
                        nc.tensor.matmul(tbp[:], xbT[:],
                                         wc_s[:, (l + 1) * TW:(l + 2) * TW],
                                         start=True, stop=True)
                        tbs = sb.tile([128, TW], BF16, tag="tabs2")
                        nc.vector.tensor_copy(tbs[:], tbp[:])
                        nc.sync.dma_start(
                            tabl[l + 1][bass.ds(i * 128, 128), :], tbs[:])
                    else:
                        x3s = sb.tile([128, 128], F32, tag="x3s")
                        nc.vector.tensor_copy(x3s[:], xtp[:])
                        nc.sync.dma_start(d_x3[:, bass.ds(i * 128, 128)], x3s[:])
                tc.strict_bb_all_engine_barrier()
                if l < L - 1:
                    nc.gpsimd.collective_compute(
                        "AllGather", mybir.AluOpType.bypass,
                        ins=[tabl[l + 1][:]], outs=[tabg[l + 1][:]],
                        replica_groups=[list(range(NCORES))])
                    tc.strict_bb_all_engine_barrier()

            # ---- phase 3: MLP ----
            x3_s = cp.tile([128, NPAD], F32, tag="x3")
            nc.sync.dma_start(x3_s[:], d_x3[:])
            orow = cp.tile([1, NPAD], F32, tag="orow")
            nchunk = (NPAD + 511) // 512
            for i in range(nchunk):
                base = i * 512
                cnt = min(512, NPAD - base)
                hp = ps1.tile([32, 512], F32, tag="xtp")
                nc.tensor.matmul(hp[:, :cnt], w1a_s[:],
                                 x3_s[:, base:base + cnt], start=True, stop=False)
                nc.tensor.matmul(hp[:, :cnt], w1b_s[:],
                                 x0_s[:, base:base + cnt], start=False, stop=True)
                h1 = sb.tile([32, 512], F32, tag="h1")
                nc.scalar.activation(h1[:, :cnt], hp[:, :cnt], AF.Relu,
                                     bias=b1_s[:])
                op_ = ps1.tile([1, 512], F32, tag="xtp")
                nc.tensor.matmul(op_[:, :cnt], w2_s[:], h1[:32, :cnt],
                                 start=True, stop=True)
                nc.scalar.activation(orow[0:1, base:base + cnt], op_[:, :cnt],
                                     AF.Sigmoid, bias=b2_s[:])
            nc.sync.dma_start(d_out[:].rearrange("n one -> one n"),
                              orow[0:1, :NPC])
    nc.compile()
    return nc


def kernel(**inputs):
    maps, TLO, THI = _host_prep(**inputs)
    key = (TLO, THI)
    if key not in _cache:
        _cache[key] = _build(TLO, THI)
    nc = _cache[key]
    results = _run_spmd(nc, maps)
    out = np.concatenate([results[k]["out"] for k in range(NCORES)], axis=0)
    return out


def run_traced(**inputs):
    """Returns (out, best_ns) — min wall-clock over repeated steady-state runs."""
    import time
    maps, TLO, THI = _host_prep(**inputs)
    key = (TLO, THI)
    if key not in _cache:
        _cache[key] = _build(TLO, THI)
    nc = _cache[key]
    results = _run_spmd(nc, maps)  # warm (compile)
    best = None
    for _ in range(3):
        t0 = time.perf_counter()
        results = _run_spmd(nc, maps)
        dt = time.perf_counter() - t0
        best = dt if best is None else min(best, dt)
    out = np.concatenate([results[k]["out"] for k in range(NCORES)], axis=0)
    return out, int(best * 1e9)
